# revision 13
# baseline (speedup 1.0000x reference)
"""MoE branch-routing kernel for Trainium2 (8 NeuronCores).

Strategy: expert-parallel with host-side routing. The batch is grouped by
`command` (4 experts); each expert's rows are split across 2 of the 8 cores.
Every core runs the same static SPMD program: a dense 3-layer MLP
(relu(xW1+b1) -> relu(hW2+b2) -> tanh(hW3+b3)) over C rows with ITS OWN
expert's weights delivered via its per-core input map. Activations stay in
[feature, row] layout on-chip so no transposes are needed; the host
transposes x in / y out and scatters rows back to their original positions.

v2: matmul operands are bf16 (fp32 PSUM accumulation; measured end-to-end
max rel err ~3.3e-3 vs the fp32 reference, well inside the 2e-2 gate).
bf16 halves weight/x DMA and SBUF, enables FWL weight loads (LDWEIGHTS
fully hidden behind the matmul stream at any clock), and draws less PE
power (less P0 downclock exposure). Relu layers run on the Vector engine
as a fused (x+bias, max 0) tensor_scalar so the Scalar engine only does
the L3 tanh; this shortens the block-0 k-outer ACT chain that stalled L2.
"""

import numpy as np
import ml_dtypes

B, D_IN, H, D_OUT, E = 16384, 512, 1024, 512, 4
N_CORES = 8
CORES_PER_EXPERT = N_CORES // E
KI, KH, MO = D_IN // 128, H // 128, D_OUT // 128  # 4, 8, 4

_CACHE = {}


def _make_blocks(C):
    """Split C rows into matmul moving-dim blocks <=512, preferring >=256
    (full PE rate needs a long moving dim; PSUM bank caps fp32 out at 512)."""
    blocks, r = [], C
    while r > 0:
        if r >= 768 or r <= 512:
            nb = min(512, r)
        else:  # 512 < r < 768: split evenly so both pieces are >= 256
            nb = (r // 2 + 31) // 32 * 32
        blocks.append(nb)
        r -= nb
    return blocks


def _build_program(C):
    import concourse.tile as tile
    from concourse import bacc, mybir

    f32, bf16 = mybir.dt.float32, mybir.dt.bfloat16
    AFT = mybir.ActivationFunctionType
    ALU = mybir.AluOpType

    nc = bacc.Bacc("TRN2", target_bir_lowering=False, debug=False,
                   num_devices=N_CORES)
    xT_d = nc.dram_tensor("xT", [D_IN, C], bf16, kind="ExternalInput").ap()
    w1_d = nc.dram_tensor("w1", [D_IN, H], bf16, kind="ExternalInput").ap()
    w2_d = nc.dram_tensor("w2", [H, H], bf16, kind="ExternalInput").ap()
    w3_d = nc.dram_tensor("w3", [H, D_OUT], bf16, kind="ExternalInput").ap()
    b1_d = nc.dram_tensor("b1t", [128, KH], f32, kind="ExternalInput").ap()
    b2_d = nc.dram_tensor("b2t", [128, KH], f32, kind="ExternalInput").ap()
    b3_d = nc.dram_tensor("b3t", [128, MO], f32, kind="ExternalInput").ap()
    yT_d = nc.dram_tensor("yT", [D_OUT, C], f32, kind="ExternalOutput").ap()

    x_re = xT_d.rearrange("(k p) c -> p k c", p=128)
    w1_re = w1_d.rearrange("(k p) h -> p k h", p=128)
    w2_re = w2_d.rearrange("(k p) h -> p k h", p=128)
    w3_re = w3_d.rearrange("(k p) h -> p k h", p=128)
    y_re = yT_d.rearrange("(m p) c -> p m c", p=128)

    blocks = _make_blocks(C)

    block_off = []
    n0 = 0
    for nb in blocks:
        block_off.append(n0)
        n0 += nb

    with tile.TileContext(nc) as tc:
        with tc.tile_pool(name="wpool", bufs=1) as wpool, \
             tc.tile_pool(name="xpool", bufs=3) as xpool, \
             tc.tile_pool(name="hpool", bufs=2) as hpool, \
             tc.tile_pool(name="ypool", bufs=2) as ypool, \
             tc.tile_pool(name="psum", bufs=8, space="PSUM") as psum:

            xtiles = {}

            def load_x(b):
                if b >= len(blocks):
                    return
                xt = xpool.tile([128, KI, 512], bf16, name="x")
                nb, o = blocks[b], block_off[b]
                nc.sync.dma_start(xt[:, :, :nb], x_re[:, :, o:o + nb])
                xtiles[b] = xt

            # HAM clock-gate warm-up: dummy matmuls on a zeroed tile keep
            # the PE busy from ~4.5us so the clock is already at full rate
            # when the real stream starts; their output lands in a psum
            # slot nothing reads, and they release it before block 0
            # claims the 8th PSUM bank. 10 iterations (~3.2us cold) is
            # enough to trip the ~3.4us HAM busy window once the real
            # stream follows on without a gap.
            warm = wpool.tile([128, 384], bf16, name="warm", bufs=1)
            nc.gpsimd.memset(warm[:], 0.0)
            # Dummy activation pre-loads the ACT spline table set during
            # the startup DMA window, so neither the block-0 relus (odd m
            # run on ScalarE) nor the first tanh pay the ~1.5-2.7us
            # ACT_TABLE_LOAD on the critical path.
            actwarm = wpool.tile([128, 1], f32, name="actwarm", bufs=1)
            nc.scalar.activation(actwarm[:], warm[:, :1], AFT.Tanh)
            # 14 iterations bridge from preamble-end (~6.6us) past the
            # slowest core's first-chunk DMA (~11.3us): a shorter warm-up
            # leaves a PE idle hole that resets the HAM busy window and
            # the real stream then crawls at the cold clock.
            wps = psum.tile([128, 512], f32, name="ps")
            for _ in range(14):
                nc.tensor.matmul(wps[:, :384], lhsT=warm[:, :128],
                                 rhs=warm[:], start=True, stop=True)

            # All bulk DMAs go on the sync (SP) HWDGE queue - the only fast
            # one (scalar HWDGE and gpsimd SWDGE measured far slower). The
            # DGE runs up to 8 DMAs concurrently, so concurrency IS the
            # bandwidth and coarse chunks win. W1/x0 interleaved ahead of
            # W2/W3; block-0 L1 runs k-outer so the first matmuls only
            # need the first W1/x0 chunk pair.
            w1sb = wpool.tile([128, KI, H], bf16)
            xt0 = xpool.tile([128, KI, 512], bf16, name="x")
            nb0 = blocks[0]
            for k in range(KI):
                nc.sync.dma_start(w1sb[:, k, :], w1_re[:, k, :])
                nc.sync.dma_start(xt0[:, k, :nb0], x_re[:, k, 0:nb0])
            xtiles[0] = xt0
            b1sb = wpool.tile([128, KH], f32)
            nc.sync.dma_start(b1sb[:], b1_d[:])
            w2sb = wpool.tile([128, KH, H], bf16)
            for k in range(KH):
                nc.sync.dma_start(w2sb[:, k, :], w2_re[:, k, :])
            b2sb = wpool.tile([128, KH], f32)
            nc.sync.dma_start(b2sb[:], b2_d[:])
            b3sb = wpool.tile([128, MO], f32)
            nc.sync.dma_start(b3sb[:], b3_d[:])
            w3sb = wpool.tile([128, KH, D_OUT], bf16)
            nc.sync.dma_start(w3sb[:], w3_re[:])
            load_x(1)
            load_x(2)

            def relu(dst, src, bias):
                # DVE fused (src + bias) max 0 -> keeps ScalarE free for
                # the tanh and halves the block-0 ACT chain latency.
                nc.vector.tensor_scalar(dst, src, bias, 0.0,
                                        ALU.add, ALU.max)

            for b, nb in enumerate(blocks):
                n0 = block_off[b]
                load_x(b + 3)
                xt = xtiles.pop(b)

                # L1: h1 = relu(x @ W1 + b1), laid out [H, rows].
                # Block 0 runs k-outer so the first matmuls only need the
                # first W1/x0 chunk pair; later blocks run m-outer so each
                # psum drains right after its accumulation group.
                h1 = []
                if b == 0:
                    # k-outer: 8 matmuls per W1/x chunk pair, matching the
                    # chunk DMA arrival rate. All 8 psums finish together,
                    # so the relus alternate DVE/ScalarE to halve the
                    # chain latency before L2's first m-group consumes h1.
                    pts = [psum.tile([128, 512], f32, name="ps")
                           for _ in range(KH)]
                    for k in range(KI):
                        for m in range(KH):
                            nc.tensor.matmul(
                                pts[m][:, :nb],
                                lhsT=w1sb[:, k, m * 128:(m + 1) * 128],
                                rhs=xt[:, k, :nb],
                                start=(k == 0), stop=(k == KI - 1))
                    for m in range(KH):
                        ht = hpool.tile([128, 512], bf16, name=f"h1_{m}")
                        relu(ht[:, :nb], pts[m][:, :nb], b1sb[:, m:m + 1])
                        h1.append(ht)
                else:
                    for m in range(KH):
                        pt = psum.tile([128, 512], f32, name="ps")
                        for k in range(KI):
                            nc.tensor.matmul(
                                pt[:, :nb],
                                lhsT=w1sb[:, k, m * 128:(m + 1) * 128],
                                rhs=xt[:, k, :nb],
                                start=(k == 0), stop=(k == KI - 1))
                        ht = hpool.tile([128, 512], bf16, name=f"h1_{m}")
                        relu(ht[:, :nb], pt[:, :nb], b1sb[:, m:m + 1])
                        h1.append(ht)

                # L2: h2 = relu(h1 @ W2 + b2)
                h2 = []
                for m in range(KH):
                    pt = psum.tile([128, 512], f32, name="ps")
                    for k in range(KH):
                        nc.tensor.matmul(
                            pt[:, :nb],
                            lhsT=w2sb[:, k, m * 128:(m + 1) * 128],
                            rhs=h1[k][:, :nb],
                            start=(k == 0), stop=(k == KH - 1))
                    ht = hpool.tile([128, 512], bf16, name=f"h2_{m}")
                    relu(ht[:, :nb], pt[:, :nb], b2sb[:, m:m + 1])
                    h2.append(ht)

                # L3: y = tanh(h2 @ W3 + b3), DMA out per m-chunk
                for m in range(MO):
                    pt = psum.tile([128, 512], f32, name="ps")
                    for k in range(KH):
                        nc.tensor.matmul(
                            pt[:, :nb],
                            lhsT=w3sb[:, k, m * 128:(m + 1) * 128],
                            rhs=h2[k][:, :nb],
                            start=(k == 0), stop=(k == KH - 1))
                    yt = ypool.tile([128, 512], f32, name=f"y{m}")
                    nc.scalar.activation(yt[:, :nb], pt[:, :nb], AFT.Tanh,
                                         bias=b3sb[:, m:m + 1])
                    if b == len(blocks) - 1 and m == MO - 1:
                        # Final output chunk rides the scalar ring: its
                        # issue isn't queued behind the sync ring's m0-m2
                        # stores, shortening the end-of-kernel drain.
                        nc.scalar.dma_start(y_re[:, m, n0:n0 + nb],
                                            yt[:, :nb])
                    else:
                        nc.sync.dma_start(y_re[:, m, n0:n0 + nb],
                                          yt[:, :nb])

    nc.compile()
    return nc


def _prepare(x, command, W1, b1, W2, b2, W3, b3):
    """Route rows to cores and build the per-core input maps.

    Returns (nc, in_maps, core_rows, nrows)."""
    bf16 = ml_dtypes.bfloat16
    x = np.ascontiguousarray(np.asarray(x, dtype=np.float32))
    command = np.asarray(command).astype(np.int64)
    b1 = np.asarray(b1, dtype=np.float32)
    b2 = np.asarray(b2, dtype=np.float32)
    b3 = np.asarray(b3, dtype=np.float32)

    nrows = x.shape[0]
    order = np.argsort(command, kind="stable")
    counts = np.bincount(command, minlength=E)
    starts = np.concatenate([[0], np.cumsum(counts)])

    # Static per-core row capacity, shared by all cores (one SPMD program).
    C = int(-(-int(counts.max()) // CORES_PER_EXPERT))
    C = max(256, -(-C // 32) * 32)

    if C not in _CACHE:
        _CACHE[C] = _build_program(C)
    nc = _CACHE[C]

    xT = np.ascontiguousarray(x.T.astype(bf16))  # [D_IN, B] bf16
    in_maps = []
    core_rows = []
    for e in range(E):
        rows_e = order[starts[e]:starts[e + 1]]
        per = -(-max(len(rows_e), 1) // CORES_PER_EXPERT)
        b1t = np.ascontiguousarray(b1[e].reshape(KH, 128).T)
        b2t = np.ascontiguousarray(b2[e].reshape(KH, 128).T)
        b3t = np.ascontiguousarray(b3[e].reshape(MO, 128).T)
        w1c = np.ascontiguousarray(np.asarray(W1[e], dtype=np.float32)
                                   .astype(bf16))
        w2c = np.ascontiguousarray(np.asarray(W2[e], dtype=np.float32)
                                   .astype(bf16))
        w3c = np.ascontiguousarray(np.asarray(W3[e], dtype=np.float32)
                                   .astype(bf16))
        for h in range(CORES_PER_EXPERT):
            rows = rows_e[h * per:(h + 1) * per]
            xTc = np.zeros((D_IN, C), dtype=bf16)
            if len(rows):
                xTc[:, :len(rows)] = xT[:, rows]
            in_maps.append({
                "xT": xTc,
                "w1": w1c, "w2": w2c, "w3": w3c,
                "b1t": b1t, "b2t": b2t, "b3t": b3t,
            })
            core_rows.append(rows)

    return nc, in_maps, core_rows, nrows


def _gather(results, core_rows, nrows):
    out = np.empty((nrows, D_OUT), dtype=np.float32)
    for c in range(N_CORES):
        rows = core_rows[c]
        if len(rows):
            out[rows] = results[c]["yT"][:, :len(rows)].T
    return out


def kernel(x, command, W1, b1, W2, b2, W3, b3):
    from concourse.bass_utils import run_bass_kernel_spmd

    nc, in_maps, core_rows, nrows = _prepare(
        x, command, W1, b1, W2, b2, W3, b3)
    res = run_bass_kernel_spmd(nc, in_maps, list(range(N_CORES)))
    return _gather(res.results, core_rows, nrows)
